# revision 6
# baseline (speedup 1.0000x reference)
"""2-layer GCN encoder (PyG GCNConv semantics) on 8 Trainium2 NeuronCores.

Key design points:
  - The layer-1 message table q~ = dis*(x@W1) is computed on HOST (x@W1 is
    3% of total FLOPs) and uploaded replicated to every core's HBM, so
    layer 1 needs NO AllGather (was 284us).
  - Layer 1 is dest-sharded: each core aggregates its own 98 dest tiles
    with the S-matmul (one-hot routing) trick from the replicated q~ table
    (4 int16 gather banks, budget 256 per (tile, bank)).
  - Layer 2 is SOURCE-sharded: each core gathers messages from its LOCAL
    h~ shard (12544 rows -> single int16 bank), computes partial sums for
    all 784 global dest tiles (budget 128 = one 128-slot chunk per tile),
    writes a bf16 partial table [8*128, 98*128] and runs a ReduceScatter
    (costs ~95us vs 284us for the AllGather it replaces).
  - Output: dis scaling is folded into a PE "transpose" matmul against a
    host-built diag(dis) matrix; @W2 runs with W2 stationary and 512-wide
    moving operands; b2 is preloaded into PSUM via a K=1 ones-matmul.
  - Epilogues batched over tile groups; relu + PSUM->SBUF copies run on the
    (otherwise idle) Activation engine.

Host permutation: nodes are snake-dealt by in-degree over the 784 global
tiles, then repaired by in-core swaps so every (src-core, dest-tile) edge
count is <= 128 (which implies every layer-1 (dest-tile, src-bank) count
is <= 256 since a bank is exactly two cores).
"""
import os
import sys

for _p in ("/opt/trn_rl_repo/concourse", "/opt/trn_rl_repo"):
    if _p not in sys.path:
        sys.path.insert(0, _p)

import numpy as np
import ml_dtypes

N = 100000
E = 640000
IN = 16
F = 128
NCORES = 8
TPC = 98                  # dest tiles per core
GT = NCORES * TPC         # 784 global tiles
SHARD = TPC * 128         # 12544 nodes per core
NP = NCORES * SHARD       # 100352 padded nodes
NBANKS = 4
BANK = NP // NBANKS       # 25088 rows per layer-1 gather bank
LTB1 = 256                # layer-1 slot budget per (tile, bank)
G1 = 4                    # layer-1 tiles per gather group
GROUPS1 = [(g * G1, min(G1, TPC - g * G1)) for g in range((TPC + G1 - 1) // G1)]
B1BASE = [0]
for _, _nt in GROUPS1:
    B1BASE.append(B1BASE[-1] + NBANKS * _nt * LTB1)
S1 = B1BASE[-1]           # 100352 layer-1 slots per core
LTB2 = 128                # layer-2 slot budget per (src-core, dest-tile)
S2 = GT * LTB2            # 100352 layer-2 slots per core
G2 = 8                    # layer-2 dest tiles per gather group
# layer-2 tile chunks: each gets its own partial tensor + ReduceScatter so
# the RS chain overlaps later chunks' compute and the final phase
CHUNKS = [(0, 24), (24, 22), (46, 22), (68, 18), (86, 12)]
RS_LEAD = 2               # issue RS_k after chunk k+RS_LEAD's gathers
GF = 4                    # final-phase tiles per group (W2 moving dim 512)


def _subgroups(cnt, g):
    return [(i * g, min(g, cnt - i * g)) for i in range((cnt + g - 1) // g)]
BF = ml_dtypes.bfloat16

_CACHE = {}


# ---------------------------------------------------------------------------
# device program
# ---------------------------------------------------------------------------

def _build_device():
    from concourse import bacc, tile, mybir

    BF16 = mybir.dt.bfloat16
    F32 = mybir.dt.float32
    I16 = mybir.dt.int16

    nc = bacc.Bacc(None, target_bir_lowering=False, num_devices=NCORES,
                   name="gcnv2", num_swdge_queues=2)

    qtab = nc.declare_dram_parameter("qtab", [NP, F], BF16, isOutput=False)
    qself = nc.declare_dram_parameter("qself", [128, TPC * F], BF16,
                                      isOutput=False)
    disc = nc.declare_dram_parameter("disc", [128, TPC], F32, isOutput=False)
    disc2 = nc.declare_dram_parameter("disc2", [128, TPC], F32,
                                      isOutput=False)
    w2p = nc.declare_dram_parameter("w2p", [F, F], BF16, isOutput=False)
    b2r = nc.declare_dram_parameter("b2r", [1, F], BF16, isOutput=False)
    idx1 = nc.declare_dram_parameter("idx1", [16, S1 // 16], I16,
                                     isOutput=False)
    dloc1 = nc.declare_dram_parameter("dloc1", [128, S1 // 128], BF16,
                                      isOutput=False)
    idx2 = nc.declare_dram_parameter("idx2", [16, S2 // 16], I16,
                                     isOutput=False)
    dloc2 = nc.declare_dram_parameter("dloc2", [128, S2 // 128], BF16,
                                      isOutput=False)
    outT = nc.declare_dram_parameter("outT", [F, SHARD], BF16, isOutput=True)

    h_tab = nc.dram_tensor("h_tab", [SHARD, F], BF16)
    partials = [nc.dram_tensor(f"partial{k}", [NCORES * 128, cnt * F], BF16)
                for k, (_, cnt) in enumerate(CHUNKS)]
    aggs = [nc.dram_tensor(f"agg{k}", [128, cnt * F], BF16)
            for k, (_, cnt) in enumerate(CHUNKS)]

    groups = [list(range(NCORES))]
    qcnt = [0]

    with tile.TileContext(nc) as tc:
        with tc.tile_pool(name="sb", bufs=1) as sb:
            # ---------- persistent metadata ----------
            qself_t = sb.tile([128, TPC, F], BF16)
            disc_t = sb.tile([128, TPC], F32)
            disc2_t = sb.tile([128, TPC], F32)
            diag_t = sb.tile([128, TPC, F], BF16)
            w2_t = sb.tile([F, F], BF16)
            b2_t = sb.tile([1, F], BF16)
            idx1_t = sb.tile([128, S1 // 16], I16)
            dloc1_t = sb.tile([128, S1 // 128], BF16)
            idx2_t = sb.tile([128, S2 // 16], I16)
            dloc2_t = sb.tile([128, S2 // 128], BF16)
            hself_t = sb.tile([128, TPC, F], BF16)

            # gather metadata first so layer-1 gathers start ASAP
            nc.sync.dma_start(dloc1_t[:], dloc1[:])
            for g8 in range(8):
                nc.sync.dma_start(idx1_t[g8 * 16:(g8 + 1) * 16, :], idx1[:])
            nc.sync.dma_start(qself_t[:], qself[:])
            nc.sync.dma_start(disc_t[:], disc[:])
            nc.sync.dma_start(disc2_t[:], disc2[:])
            nc.sync.dma_start(w2_t[:], w2p[:])
            nc.sync.dma_start(b2_t[:], b2r[:])
            nc.sync.dma_start(dloc2_t[:], dloc2[:])
            for g8 in range(8):
                nc.sync.dma_start(idx2_t[g8 * 16:(g8 + 1) * 16, :], idx2[:])

            iota_t = sb.tile([128, 128], BF16)
            nc.gpsimd.iota(iota_t[:], pattern=[[1, 128]], base=0,
                           channel_multiplier=0,
                           allow_small_or_imprecise_dtypes=True)
            ones_t = sb.tile([1, 512], BF16)
            nc.gpsimd.memset(ones_t[:], 1.0)
            one128_t = sb.tile([128, 128], BF16)
            nc.gpsimd.memset(one128_t[:], 1.0)
            ident_t = sb.tile([128, 128], BF16)
            nc.gpsimd.affine_select(
                ident_t[:], one128_t[:], pattern=[[-1, 128]],
                compare_op=mybir.AluOpType.is_equal, fill=0.0,
                base=0, channel_multiplier=1)
            zb = sb.tile([128, 1], F32)
            nc.gpsimd.memset(zb[:], 0.0)

            h_tab_w = h_tab[:].rearrange("(p t) f -> p t f", p=128, t=TPC)

            # ---------- layer 1: dest-sharded from replicated q~ ----------
            with (
                tc.tile_pool(name="mb1", bufs=3) as mb1p,
                tc.tile_pool(name="s1", bufs=3) as s1p,
                tc.tile_pool(name="ep1", bufs=2) as ep1p,
                tc.tile_pool(name="pp1", bufs=2, space="PSUM") as pp1p,
            ):
                for g, (t0, nt) in enumerate(GROUPS1):
                    nch = NBANKS * nt * (LTB1 // 128)
                    mb = mb1p.tile([128, nch, F], BF16, tag="mb1")
                    s0 = B1BASE[g]
                    nidx = nt * LTB1
                    for b in range(NBANKS):
                        c0 = (s0 + b * nidx) // 16
                        nc.gpsimd.dma_gather(
                            mb[:, b * (nt * 2):(b + 1) * (nt * 2), :],
                            qtab[b * BANK:(b + 1) * BANK, :],
                            idx1_t[:, c0:c0 + nidx // 16],
                            nidx, nidx, F, queue_num=qcnt[0] % 2)
                        qcnt[0] += 1
                    S_t = s1p.tile([128, nch, 128], BF16, tag="S1")
                    nc.vector.tensor_tensor(
                        S_t[:],
                        dloc1_t[:, s0 // 128:s0 // 128 + nch]
                            .unsqueeze(-1).broadcast_to([128, nch, 128]),
                        iota_t[:].unsqueeze(1).broadcast_to([128, nch, 128]),
                        mybir.AluOpType.is_equal)
                    ps = pp1p.tile([128, nt, F], F32, tag="pp1")
                    for ti in range(nt):
                        for b in range(NBANKS):
                            for k in range(LTB1 // 128):
                                ch = b * (nt * 2) + ti * 2 + k
                                nc.tensor.matmul(
                                    ps[:, ti, :], S_t[:, ch, :], mb[:, ch, :],
                                    start=(b == 0 and k == 0),
                                    stop=(b == NBANKS - 1 and k == 1))
                    # epilogue: h~ = dis*relu(dis*(agg + q~self) + b1)
                    #             = relu(dis^2*(agg + q~self + b1/dis))
                    # (b1/dis is host-merged into the uploaded qself)
                    t1 = ep1p.tile([128, nt, F], BF16, tag="t1")
                    nc.vector.tensor_tensor(t1[:], ps[:],
                                            qself_t[:, t0:t0 + nt, :],
                                            mybir.AluOpType.add)
                    for ti in range(nt):
                        nc.scalar.activation(
                            hself_t[:, t0 + ti, :], t1[:, ti, :],
                            mybir.ActivationFunctionType.Relu,
                            bias=zb[:], scale=disc2_t[:, t0 + ti:t0 + ti + 1])
                    nc.sync.dma_start(h_tab_w[:, t0:t0 + nt, :],
                                      hself_t[:, t0:t0 + nt, :])

            # diag(dis) built on-device at the L1->L2 junction (DVE is idle
            # there); needed only in the final phase
            nc.vector.tensor_tensor(
                diag_t[:],
                ident_t[:].unsqueeze(1).broadcast_to([128, TPC, 128]),
                disc_t[:].unsqueeze(-1).broadcast_to([128, TPC, 128]),
                mybir.AluOpType.mult)

            # ---------- layer 2: source-sharded partials ----------
            def emit_rs(k):
                nc.gpsimd.collective_compute(
                    "ReduceScatter", mybir.AluOpType.add,
                    replica_groups=groups,
                    ins=[partials[k][:].opt()], outs=[aggs[k][:].opt()])

            with (
                tc.tile_pool(name="mb2", bufs=3) as mb2p,
                tc.tile_pool(name="s2", bufs=3) as s2p,
                tc.tile_pool(name="st2", bufs=3) as st2p,
                tc.tile_pool(name="pp2", bufs=2, space="PSUM") as pp2p,
            ):
                for k, (ct0, cnt) in enumerate(CHUNKS):
                    for rc in range(NCORES):
                        for (t0r, nt) in _subgroups(cnt, G2):
                            t0 = ct0 + t0r
                            gt0 = rc * TPC + t0
                            mb = mb2p.tile([128, G2, F], BF16, tag="mb2")
                            nidx = nt * LTB2
                            s0 = gt0 * LTB2
                            nc.gpsimd.dma_gather(
                                mb[:, :nt, :], h_tab[:],
                                idx2_t[:, s0 // 16:s0 // 16 + nidx // 16],
                                nidx, nidx, F, queue_num=qcnt[0] % 2)
                            qcnt[0] += 1
                            S_t = s2p.tile([128, G2, 128], BF16, tag="S2")
                            nc.vector.tensor_tensor(
                                S_t[:, :nt, :],
                                dloc2_t[:, s0 // 128:s0 // 128 + nt]
                                    .unsqueeze(-1)
                                    .broadcast_to([128, nt, 128]),
                                iota_t[:].unsqueeze(1)
                                    .broadcast_to([128, nt, 128]),
                                mybir.AluOpType.is_equal)
                            ps = pp2p.tile([128, G2, F], F32, tag="pp2")
                            for j in range(nt):
                                nc.tensor.matmul(ps[:, j, :], S_t[:, j, :],
                                                 mb[:, j, :],
                                                 start=True, stop=True)
                            st = st2p.tile([128, G2, F], BF16, tag="st2")
                            nc.scalar.activation(
                                st[:, :nt, :], ps[:, :nt, :],
                                mybir.ActivationFunctionType.Copy)
                            nc.sync.dma_start(
                                partials[k][rc * 128:(rc + 1) * 128,
                                            t0r * F:(t0r + nt) * F],
                                st[:, :nt, :])
                    # staggered ReduceScatter issue: Pool.SEQ blocks while a
                    # collective waits, so keep RS_j a couple of chunks behind
                    if k - RS_LEAD >= 0:
                        emit_rs(k - RS_LEAD)
                for k in range(len(CHUNKS) - RS_LEAD, len(CHUNKS)):
                    if k >= 0:
                        emit_rs(k)

            # ---------- final: out = dis*(agg + h~self) @ W2 + b2 ----------
            with (
                tc.tile_pool(name="fin", bufs=4) as finp,
                tc.tile_pool(name="agf", bufs=4) as agfp,
                tc.tile_pool(name="tps", bufs=3, space="PSUM") as tpsp,
                tc.tile_pool(name="ops", bufs=3, space="PSUM") as opsp,
            ):
                for k, (ct0, cnt) in enumerate(CHUNKS):
                    for (t0r, nt) in _subgroups(cnt, GF):
                        t0 = ct0 + t0r
                        aggt_t = agfp.tile([128, GF, F], BF16, tag="aggt")
                        nc.sync.dma_start(
                            aggt_t[:, :nt, :],
                            aggs[k][:, t0r * F:(t0r + nt) * F])
                        u = finp.tile([128, GF, F], BF16, tag="u")
                        nc.vector.tensor_tensor(u[:, :nt, :],
                                                aggt_t[:, :nt, :],
                                                hself_t[:, t0:t0 + nt, :],
                                                mybir.AluOpType.add)
                        tp = tpsp.tile([128, GF, F], F32, tag="tps")
                        for j in range(nt):
                            nc.tensor.matmul(tp[:, j, :], u[:, j, :],
                                             diag_t[:, t0 + j, :],
                                             start=True, stop=True)
                        g2 = finp.tile([128, GF, F], BF16, tag="g2")
                        nc.scalar.activation(g2[:, :nt, :], tp[:, :nt, :],
                                             mybir.ActivationFunctionType.Copy)
                        op = opsp.tile([128, GF * F], F32, tag="ops")
                        nc.tensor.matmul(op[:, :nt * F], b2_t[:],
                                         ones_t[:, :nt * F],
                                         start=True, stop=False)
                        nc.tensor.matmul(op[:, :nt * F], w2_t[:],
                                         g2[:, :nt, :], start=False, stop=True)
                        oc = finp.tile([128, GF * F], BF16, tag="oc")
                        nc.vector.tensor_copy(oc[:, :nt * F], op[:, :nt * F])
                        nc.sync.dma_start(outT[:, t0 * F:(t0 + nt) * F],
                                          oc[:, :nt * F])

    nc.compile()
    return nc


# ---------------------------------------------------------------------------
# persistent SPMD runner (same as v1)
# ---------------------------------------------------------------------------

class _SpmdRunner:
    def __init__(self, nc, n_cores):
        import jax
        from jax.sharding import Mesh, PartitionSpec
        from jax.experimental.shard_map import shard_map
        from concourse import bass2jax, mybir

        bass2jax.install_neuronx_cc_hook()
        self.jax = jax
        self.n_cores = n_cores
        partition_name = (nc.partition_id_tensor.name
                          if nc.partition_id_tensor else None)
        in_names, out_names, out_avals, zero_outs = [], [], [], []
        for alloc in nc.m.functions[0].allocations:
            if not isinstance(alloc, mybir.MemoryLocationSet):
                continue
            if not alloc.memorylocations:
                continue
            name = alloc.memorylocations[0].name
            if alloc.kind == "ExternalInput":
                if name != partition_name:
                    in_names.append(name)
            elif alloc.kind == "ExternalOutput":
                out_names.append(name)
                shape = tuple(alloc.tensor_shape)
                dtype = mybir.dt.np(alloc.dtype)
                out_avals.append(jax.core.ShapedArray(shape, dtype))
                zero_outs.append(np.zeros(shape, dtype))
        self.in_names, self.out_names = in_names, out_names
        self.out_avals, self.zero_outs = out_avals, zero_outs
        n_params = len(in_names)
        n_outs = len(out_avals)
        all_in = list(in_names) + list(out_names)
        if partition_name is not None:
            all_in.append(partition_name)
        donate = tuple(range(n_params, n_params + n_outs))

        def _body(*args):
            operands = list(args)
            if partition_name is not None:
                operands.append(bass2jax.partition_id_tensor())
            outs = bass2jax._bass_exec_p.bind(
                *operands,
                out_avals=tuple(out_avals),
                in_names=tuple(all_in),
                out_names=tuple(out_names),
                lowering_input_output_aliases=(),
                sim_require_finite=True,
                sim_require_nnan=True,
                nc=nc,
            )
            return tuple(outs)

        devices = jax.devices()[:n_cores]
        mesh = Mesh(np.asarray(devices), ("core",))
        in_specs = (PartitionSpec("core"),) * (n_params + n_outs)
        out_specs = (PartitionSpec("core"),) * len(out_names)
        self.fn = jax.jit(
            shard_map(_body, mesh=mesh, in_specs=in_specs,
                      out_specs=out_specs, check_rep=False),
            donate_argnums=donate, keep_unused=True)

    def run(self, in_maps):
        concat = [np.concatenate(
            [np.asarray(in_maps[c][name]) for c in range(self.n_cores)],
            axis=0) for name in self.in_names]
        zeros = [np.zeros((self.n_cores * z.shape[0], *z.shape[1:]), z.dtype)
                 for z in self.zero_outs]
        out = self.fn(*concat, *zeros)
        self.jax.block_until_ready(out)
        return [{name: np.asarray(out[i]).reshape(
                    self.n_cores, *self.out_avals[i].shape)[c]
                 for i, name in enumerate(self.out_names)}
                for c in range(self.n_cores)]


def _get_runner():
    if "runner" not in _CACHE:
        nc = _build_device()
        _CACHE["nc"] = nc
        _CACHE["runner"] = _SpmdRunner(nc, NCORES)
    return _CACHE["runner"]


# ---------------------------------------------------------------------------
# host-side preparation
# ---------------------------------------------------------------------------

def _prep(edge_index):
    """Permutation + slot packing.

    Returns dict with pos/perm, dis (by node), per-core idx/dloc arrays for
    both layers.
    """
    row = np.asarray(edge_index[0], dtype=np.int64)
    col = np.asarray(edge_index[1], dtype=np.int64)

    deg = np.bincount(col, minlength=NP).astype(np.float32) + 1.0
    dis = 1.0 / np.sqrt(deg)
    indeg = np.bincount(row, minlength=NP)

    # snake-deal nodes over the 784 global tiles by in-degree
    order = np.argsort(-indeg, kind="stable")
    k = np.arange(NP)
    rnd, pin = k // GT, k % GT
    gt_k = np.where(rnd % 2 == 0, pin, GT - 1 - pin)
    srow_k = rnd
    gt_of = np.empty(NP, dtype=np.int64)
    srow_of = np.empty(NP, dtype=np.int64)
    gt_of[order] = gt_k
    srow_of[order] = srow_k

    # src core is invariant under in-core swaps
    src_core = gt_of[col] // TPC
    load2 = np.bincount(src_core * GT + gt_of[row],
                        minlength=NCORES * GT).reshape(NCORES, GT)
    ibc = np.zeros((NP, NCORES), dtype=np.int32)
    np.add.at(ibc, (row, src_core), 1)

    if load2.max() > LTB2:
        # members per tile
        o2 = np.argsort(gt_of, kind="stable")
        bounds = np.searchsorted(gt_of[o2], np.arange(GT + 1))
        members = [list(o2[bounds[t]:bounds[t + 1]]) for t in range(GT)]
        for _ in range(20000):
            viol = np.argwhere(load2 > LTB2)
            if len(viol) == 0:
                break
            c, gt = int(viol[0][0]), int(viol[0][1])
            cc = gt // TPC
            mem = np.asarray(members[gt])
            dcand = mem[np.argsort(-ibc[mem, c], kind="stable")[:12]]
            tiles_cc = np.arange(cc * TPC, (cc + 1) * TPC)
            cand = tiles_cc[np.argsort(load2[c, tiles_cc], kind="stable")]
            cur_excess = int(np.maximum(load2[:, gt] - LTB2, 0).sum())
            done = False
            for d in dcand:
                d = int(d)
                vd = ibc[d]
                if ibc[d, c] == 0:
                    break
                for gt2 in cand:
                    gt2 = int(gt2)
                    if gt2 == gt:
                        continue
                    mem2 = np.asarray(members[gt2])
                    newgt = load2[:, gt][None, :] - vd[None, :] + ibc[mem2]
                    newg2 = load2[:, gt2][None, :] + vd[None, :] - ibc[mem2]
                    # total excess must strictly decrease; gt2 stays clean
                    nexc = (np.maximum(newgt - LTB2, 0).sum(axis=1)
                            + np.maximum(newg2 - LTB2, 0).sum(axis=1))
                    ok = nexc < cur_excess
                    if not ok.any():
                        continue
                    sel = np.flatnonzero(ok)
                    d2 = int(mem2[sel[np.argmin(
                        nexc[sel] * 1000 + newg2[sel].max(axis=1))]])
                    vd2 = ibc[d2]
                    load2[:, gt] += vd2 - vd
                    load2[:, gt2] += vd - vd2
                    members[gt].remove(d)
                    members[gt2].remove(d2)
                    members[gt].append(d2)
                    members[gt2].append(d)
                    gt_of[d], gt_of[d2] = gt2, gt
                    srow_of[d], srow_of[d2] = srow_of[d2], srow_of[d]
                    done = True
                    break
                if done:
                    break
            if not done:
                raise RuntimeError(
                    f"repair stuck: load2[{c},{gt}]={load2[c, gt]}")
        else:
            raise RuntimeError(f"repair did not converge: {load2.max()}")

    pos = (gt_of // TPC) * SHARD + srow_of * TPC + (gt_of % TPC)
    perm = np.empty(NP, dtype=np.int64)
    perm[pos] = np.arange(NP)

    prow = pos[row]
    pcol = pos[col]
    cd = prow // SHARD
    sd = prow % SHARD
    pd = sd // TPC
    td = sd % TPC
    cs = pcol // SHARD
    ss = pcol % SHARD
    bank = pcol // BANK
    bl = pcol % BANK

    # ----- layer-1 slots: key (dest core, dest tile, src bank) -----
    key1 = ((cd * TPC + td) * NBANKS + bank)
    loads1 = np.bincount(key1, minlength=NCORES * TPC * NBANKS)
    if loads1.max() > LTB1:
        raise RuntimeError(f"L1 budget overflow: {loads1.max()}")
    o1 = np.argsort(key1, kind="stable")
    k1s = key1[o1]
    starts1 = np.zeros(NCORES * TPC * NBANKS + 1, dtype=np.int64)
    np.cumsum(loads1, out=starts1[1:])
    rank1 = np.arange(E) - starts1[k1s]
    td_s = (k1s // NBANKS) % TPC
    b_s = k1s % NBANKS
    cd_s = k1s // (TPC * NBANKS)
    g1 = td_s // G1
    tin = td_s % G1
    nt1 = np.minimum(G1, TPC - g1 * G1)
    b1base = np.asarray(B1BASE[:-1], dtype=np.int64)
    slot1 = b1base[g1] + b_s * (nt1 * LTB1) + tin * LTB1 + rank1

    idx1_arr = np.zeros((NCORES, 16, S1 // 16), dtype=np.int16)
    dloc1_arr = np.full((NCORES, 128, S1 // 128), 255.0, dtype=BF)
    idx1_arr[cd_s, slot1 % 16, slot1 // 16] = bl[o1].astype(np.int16)
    dloc1_arr[cd_s, slot1 % 128, slot1 // 128] = \
        pd[o1].astype(np.float32).astype(BF)

    # ----- layer-2 slots: key (src core, global dest tile) -----
    gtd = cd * TPC + td
    key2 = cs * GT + gtd
    loads2 = np.bincount(key2, minlength=NCORES * GT)
    if loads2.max() > LTB2:
        raise RuntimeError(f"L2 budget overflow: {loads2.max()}")
    o2 = np.argsort(key2, kind="stable")
    k2s = key2[o2]
    starts2 = np.zeros(NCORES * GT + 1, dtype=np.int64)
    np.cumsum(loads2, out=starts2[1:])
    rank2 = np.arange(E) - starts2[k2s]
    gtd_s = k2s % GT
    cs_s = k2s // GT
    slot2 = gtd_s * LTB2 + rank2

    idx2_arr = np.zeros((NCORES, 16, S2 // 16), dtype=np.int16)
    dloc2_arr = np.full((NCORES, 128, S2 // 128), 255.0, dtype=BF)
    idx2_arr[cs_s, slot2 % 16, slot2 // 16] = ss[o2].astype(np.int16)
    dloc2_arr[cs_s, slot2 % 128, slot2 // 128] = \
        pd[o2].astype(np.float32).astype(BF)

    return dict(pos=pos, perm=perm, dis=dis,
                idx1=idx1_arr, dloc1=dloc1_arr,
                idx2=idx2_arr, dloc2=dloc2_arr)


def _host_tables(x, W1, b1, prep):
    """q~ table (replicated), per-core qself (with b1/dis merged), disc."""
    dis = prep["dis"]
    perm = prep["perm"]
    xp = np.zeros((NP, IN), dtype=np.float32)
    xp[:N] = x
    q_nodes = (xp @ W1) * dis[:, None]
    qtab = q_nodes[perm].astype(BF)            # [NP, F] in position order
    dis_pos = dis[perm].astype(np.float32)
    # qself rows carry q~ + b1/dis so the L1 epilogue is
    # relu(dis^2 * (agg + qself))
    qs_nodes = q_nodes + b1[None, :] / dis[:, None]
    qsp = qs_nodes[perm].astype(BF)

    qself, disc = [], []
    for c in range(NCORES):
        sl = slice(c * SHARD, (c + 1) * SHARD)
        qself.append(np.ascontiguousarray(
            qsp[sl].reshape(128, TPC * F)))
        dc = dis_pos[sl].reshape(128, TPC)
        disc.append(np.ascontiguousarray(dc))
    return qtab, qself, disc


# ---------------------------------------------------------------------------
# numpy emulation of the device dataflow (for host-prep validation)
# ---------------------------------------------------------------------------

def _emulate(x, W1, b1, W2, b2, prep):
    qtab, qself, disc = _host_tables(x, W1, b1, prep)
    qtabf = qtab.astype(np.float32)
    b1f = b1.astype(np.float32)
    W2f = W2.astype(BF).astype(np.float32)
    b2f = b2.astype(np.float32)

    # slot metadata for layer 1: per-slot (bank, tile)
    s = np.arange(S1)
    g_of = np.searchsorted(np.asarray(B1BASE), s, side="right") - 1
    off = s - np.asarray(B1BASE)[g_of]
    nt_of = np.minimum(G1, TPC - g_of * G1)
    b_of = off // (nt_of * LTB1)
    tin_of = (off % (nt_of * LTB1)) // LTB1
    t_of = g_of * G1 + tin_of

    s2v = np.arange(S2)
    gt2_of = s2v // LTB2

    h_tabs = []
    hselfs = []
    partials = np.zeros((NCORES, NCORES * 128, TPC * F), dtype=np.float32)
    for c in range(NCORES):
        idxv = prep["idx1"][c][s % 16, s // 16].astype(np.int64)
        dlv = prep["dloc1"][c][s % 128, s // 128].astype(np.float32)
        msg = qtabf[b_of * BANK + idxv]                       # [S1, F]
        valid = dlv < 128
        agg = np.zeros((TPC, 128, F), dtype=np.float32)
        np.add.at(agg, (t_of[valid], dlv[valid].astype(np.int64)), msg[valid])
        agg = agg.transpose(1, 0, 2)                          # [p, t, F]
        qs = qself[c].reshape(128, TPC, F).astype(np.float32)
        dc = disc[c][:, :, None]
        hs = np.maximum((dc * dc) * (agg + qs), 0.0).astype(BF)  # [p, t, F]
        hselfs.append(hs.astype(np.float32))
        h_tabs.append(hs.reshape(SHARD, F).astype(np.float32))

    for c in range(NCORES):
        idxv = prep["idx2"][c][s2v % 16, s2v // 16].astype(np.int64)
        dlv = prep["dloc2"][c][s2v % 128, s2v // 128].astype(np.float32)
        msg = h_tabs[c][idxv]
        valid = dlv < 128
        part = np.zeros((GT, 128, F), dtype=np.float32)
        np.add.at(part, (gt2_of[valid], dlv[valid].astype(np.int64)),
                  msg[valid])
        # partial layout [rc*128 + p, t*F + f]
        part = part.reshape(NCORES, TPC, 128, F).transpose(0, 2, 1, 3)
        partials[c] = part.astype(BF).astype(np.float32).reshape(
            NCORES * 128, TPC * F)

    aggsum = partials.sum(axis=0)                             # bf16-summed ~
    outs = []
    for c in range(NCORES):
        agg2 = aggsum[c * 128:(c + 1) * 128].reshape(128, TPC, F)
        u = (agg2 + hselfs[c]).astype(BF).astype(np.float32)
        dc = disc[c].astype(np.float32)[:, :, None]
        g2v = (u * dc).astype(BF).astype(np.float32)
        o = np.einsum("ptf,fo->pto", g2v, W2f) + b2f[None, None, :]
        outs.append(o)                                        # [p, t, F]
    # assemble
    pos = prep["pos"][:N]
    c = pos // SHARD
    ssh = pos % SHARD
    p = ssh // TPC
    t = ssh % TPC
    out = np.empty((N, F), dtype=np.float32)
    for cc in range(NCORES):
        m = c == cc
        out[m] = outs[cc][p[m], t[m], :]
    return out


# ---------------------------------------------------------------------------
# entry point
# ---------------------------------------------------------------------------

def kernel(x, W1, b1, W2, b2, edge_index, _emulate_only=False):
    x = np.asarray(x, dtype=np.float32)
    W1 = np.asarray(W1, dtype=np.float32)
    b1 = np.asarray(b1, dtype=np.float32)
    W2 = np.asarray(W2, dtype=np.float32)
    b2 = np.asarray(b2, dtype=np.float32)
    edge_index = np.asarray(edge_index)

    prep = _prep(edge_index)
    if _emulate_only:
        return _emulate(x, W1, b1, W2, b2, prep)

    qtab, qself, disc = _host_tables(x, W1, b1, prep)
    w2p = W2.astype(BF)
    b2r = b2[None, :].astype(BF)

    in_maps = []
    for c in range(NCORES):
        in_maps.append({
            "qtab": qtab,
            "qself": qself[c],
            "disc": disc[c],
            "disc2": disc[c] * disc[c],
            "w2p": w2p,
            "b2r": b2r,
            "idx1": prep["idx1"][c],
            "dloc1": prep["dloc1"][c],
            "idx2": prep["idx2"][c],
            "dloc2": prep["dloc2"][c],
        })

    r = _get_runner()
    res = r.run(in_maps)

    pos = prep["pos"][:N]
    c = pos // SHARD
    ssh = pos % SHARD
    p = ssh // TPC
    t = ssh % TPC
    colix = t * 128 + p
    out = np.empty((N, F), dtype=np.float32)
    for cc in range(NCORES):
        m = c == cc
        out[m] = res[cc]["outT"][:, colix[m]].T
    return out
